# revision 1
# baseline (speedup 1.0000x reference)
"""Bounding-box kernel for Trainium2 (Bass/Tile), 8-core SPMD.

Problem: mask [128, 1, 512, 512] f32 -> bbox [128, 4] int32
  (y_min, x_min, y_max, x_max) of the region where mask >= 0.5,
  with (0, 0, H, W) when a row/col has no hit.

Strategy (per core, 16 images, single qSync HWDGE queue):
  - DMA each image [512, 512] as one [128, 4, 512] tile (partition p
    holds rows 4p..4p+3 -> contiguous 8KB descriptors, the per-engine
    throughput sweet spot: ~26.5 GB/s x 16 engines ~ 424 GB/s).
  - Threshold: ACT computes h = Relu(x*2^25 - (2^24-1)) in bf16, which
    is exactly 0 iff x < 0.5 and >= 1 otherwise (exact for every f32).
  - Column extents: one-hot [128, 16] lhsT matmuls accumulate per-image
    column hit-mass into PSUM [16, 512] (partition = image); gpsimd
    pre-adds block pairs to halve the matmul count; then compare/mul/
    reduce on DVE.
  - Row extents stay in [128, *] space: rowmax over W of h (bf16)
    -> [128, 64] (col = i*4 + b, image row r = 4p + b), compare, mul
    by index consts, reduce over b -> [128, 16] (col = image), one PE
    transpose -> [16, 128] PSUM, full-partition reduce. This chain
    runs on gpsimd at the tail, in parallel with the X chain on DVE.
  - Last image: two [128, 2, 512] half loads (4KB descriptors) so the
    final arrival -> answer chain is short.
"""

import numpy as np
import ml_dtypes
from contextlib import ExitStack

import concourse.bass as bass
import concourse.bacc as bacc
import concourse.tile as tile
import concourse.mybir as mybir
from concourse.bass_utils import run_bass_kernel_spmd

N_CORES = 8
N, H, W = 128, 512, 512
NPC = N // N_CORES          # images per core = 16
P = 128                     # SBUF partitions
NBLK = H // P               # 4 row blocks per image
F32 = mybir.dt.float32
BF16 = mybir.dt.bfloat16
I32 = mybir.dt.int32

# Relu(x * 2^25 - (2^24 - 1)) == 0 iff x < 0.5, >= 1 iff x >= 0.5, exact
# for EVERY f32 x: x*2^25 is exact (power-of-2 scale); for x < 0.5,
# x*2^25 <= 2^24 - 1 so the true sum is <= 0 (rounding is monotone, 0 is
# representable); for x >= 0.5 the true sum is >= 1 and rounds to >= 1.
ACT_SCALE = float(2**25)
ACT_BIAS = float(1 - 2**24)

TRACE = False               # test.py sets True to capture a HW profile
LAST_RESULTS = None         # BassKernelResults of the last run

_compiled = None


def _build_nc():
    nc = bacc.Bacc(
        "TRN2", target_bir_lowering=False, debug=False, num_devices=N_CORES
    )
    mask_d = nc.dram_tensor("mask", [NPC * H, W], F32, kind="ExternalInput").ap()
    oneh_d = nc.dram_tensor("onehot", [P, NPC * NPC], BF16, kind="ExternalInput").ap()
    # packed f32 consts: ident [0:128] | ylo [128:192] | yhi [192:256]
    pack_d = nc.dram_tensor("cpack", [P, 2 * P], F32, kind="ExternalInput").ap()
    xlo_d = nc.dram_tensor("xlo_const", [NPC, W], F32, kind="ExternalInput").ap()
    xhi_d = nc.dram_tensor("xhi_const", [NPC, W], F32, kind="ExternalInput").ap()
    bbox_d = nc.dram_tensor("bbox", [NPC, 4], I32, kind="ExternalOutput").ap()

    with tile.TileContext(nc) as tc, ExitStack() as ctx:
        consts = ctx.enter_context(tc.tile_pool(name="consts", bufs=1))
        xpool = ctx.enter_context(tc.tile_pool(name="x", bufs=4))
        hpool = ctx.enter_context(tc.tile_pool(name="h", bufs=6))
        hspool = ctx.enter_context(tc.tile_pool(name="hs", bufs=4))
        lastpool = ctx.enter_context(tc.tile_pool(name="last", bufs=2))
        small = ctx.enter_context(tc.tile_pool(name="small", bufs=1))
        scratch = ctx.enter_context(tc.tile_pool(name="scratch", bufs=2))
        psum = ctx.enter_context(tc.tile_pool(name="psum", bufs=1, space="PSUM"))

        # pin const loads to the start of the schedule, all on the SYNC
        # queue: the scalar queue stays completely idle, so the DMA
        # queue-manager engine (which also carries 1/16 of the mask and
        # is the stream straggler) never juggles two descriptor streams.
        # Packed shapes keep the const descriptor count tiny. The tile
        # scheduler otherwise sinks tail-only consts next to their
        # consumers, adding their DMA latency to the tail.
        with tc.high_priority():
            oneh = consts.tile([P, NPC * NPC], BF16)
            nc.scalar.dma_start(out=oneh[:], in_=oneh_d)
            cpack = consts.tile([P, 2 * P], F32)
            nc.scalar.dma_start(out=cpack[:], in_=pack_d)
            xlo_c = consts.tile([NPC, W], F32)
            nc.scalar.dma_start(out=xlo_c[:], in_=xlo_d)
            xhi_c = consts.tile([NPC, W], F32)
            nc.scalar.dma_start(out=xhi_c[:], in_=xhi_d)
            act_bias = consts.tile([P, 1], F32)
            nc.vector.memset(act_bias[:], ACT_BIAS)
        ident = cpack[:, 0:P]
        ylo_c = cpack[:, P:P + NPC * NBLK]
        yhi_c = cpack[:, P + NPC * NBLK:2 * P]

        # rowmax[p, i*4 + b]: any-hit indicator per image row r = 4p + b.
        # Images 0-13: DVE max over w of h. Images 14/15: ACT accum_out
        # row SUMS (sum of relu-mass: 0 iff no hit) — frees DVE's tail.
        # Both are 0 iff no hit and >= 1 otherwise, so one is_ge works.
        rowmax = small.tile([P, NPC * NBLK], F32)
        rowmax_v = rowmax.rearrange("p (i b) -> p i b", i=NPC)
        cnt_ps = psum.tile([NPC, W], F32)    # per-image column hit-mass
        tpsL = psum.tile([NPC, P], F32)      # transposed y-lo stage
        tpsH = psum.tile([NPC, P], F32)      # transposed y-hi stage

        # images 0-13 as 7 DUAL-image DMA instructions (256 x 8KB
        # descriptors each): halves the mask instruction count — the
        # queue-manager DMA engine (the stream straggler) pays a
        # per-instruction completion overhead, so fewer instructions
        # pull in the stream tail
        for t in range(7):
            x = xpool.tile([P, 2, NBLK, W], F32, tag="x")
            nc.sync.dma_start(
                out=x[:],
                in_=mask_d[2 * t * H:(2 * t + 2) * H, :]
                .rearrange("(i p b) w -> p i b w", i=2, p=P),
            )
            h = hpool.tile([P, 2, NBLK, W], BF16, tag="h")
            for j in range(2):
                nc.scalar.activation(
                    h[:, j], x[:, j], mybir.ActivationFunctionType.Relu,
                    bias=act_bias[:], scale=ACT_SCALE,
                )
                nc.vector.tensor_reduce(
                    out=rowmax_v[:, 2 * t + j, :], in_=h[:, j],
                    axis=mybir.AxisListType.X, op=mybir.AluOpType.max,
                )
            if t < 6:
                # pre-sum block pairs on gpsimd, both images in one op:
                # halves the PE matmul count (hit-mass stays 0 iff no hit)
                h_v = h.rearrange("p i (m q) w -> p i m q w", q=2)
                hs = hspool.tile([P, 2, 2, W], BF16)
                nc.gpsimd.tensor_add(hs[:], h_v[:, :, :, 0, :], h_v[:, :, :, 1, :])
                for j in range(2):
                    lhsT = oneh[:, (2 * t + j) * NPC:(2 * t + j + 1) * NPC]
                    for m in range(2):
                        nc.tensor.matmul(
                            cnt_ps[:, :], lhsT, hs[:, j, m, :],
                            start=(t == 0 and j == 0 and m == 0), stop=False,
                        )
            else:
                # last dual (images 12,13): direct matmuls — the 4us
                # gpsimd hop sits on the tail critical path (PE program
                # order makes images 14/15's matmuls queue behind it)
                for j in range(2):
                    lhsT = oneh[:, (2 * t + j) * NPC:(2 * t + j + 1) * NPC]
                    for b in range(NBLK):
                        nc.tensor.matmul(
                            cnt_ps[:, :], lhsT, h[:, j, b, :],
                            start=False, stop=False,
                        )

        # image 14: single-image DMA into half a dual tile (stays on the
        # xpool ring so its trigger is paced like the duals), per-block
        # acts with accum_out row sums (no DVE rowmax), direct matmuls
        i = NPC - 2
        x = xpool.tile([P, 2, NBLK, W], F32, tag="x")
        nc.sync.dma_start(
            out=x[:, 0],
            in_=mask_d[i * H:(i + 1) * H, :].rearrange("(p b) w -> p b w", p=P),
        )
        h = hpool.tile([P, 2, NBLK, W], BF16, tag="h")
        lhsT = oneh[:, i * NPC:(i + 1) * NPC]
        for b in range(NBLK):
            nc.scalar.activation(
                h[:, 0, b:b + 1, :], x[:, 0, b:b + 1, :],
                mybir.ActivationFunctionType.Relu,
                bias=act_bias[:], scale=ACT_SCALE,
                accum_out=rowmax_v[:, i, b:b + 1],
            )
            nc.tensor.matmul(
                cnt_ps[:, :], lhsT, h[:, 0, b, :],
                start=False, stop=False,
            )

        # image 15: two half loads so its compute chain starts while
        # the second half is still in flight; per-block acts + accum
        i = NPC - 1
        lhsT = oneh[:, i * NPC:(i + 1) * NPC]
        for u in range(2):
            x = lastpool.tile([P, 2, W], F32, tag="xh")
            nc.sync.dma_start(
                out=x[:],
                in_=mask_d[i * H:(i + 1) * H, :]
                .rearrange("(p b) w -> p b w", p=P)[:, 2 * u:2 * u + 2, :],
            )
            h = lastpool.tile([P, 2, W], BF16, tag="hh")
            for b in range(2):
                nc.scalar.activation(
                    h[:, b:b + 1, :], x[:, b:b + 1, :],
                    mybir.ActivationFunctionType.Relu,
                    bias=act_bias[:], scale=ACT_SCALE,
                    accum_out=rowmax_v[:, i, 2 * u + b:2 * u + b + 1],
                )
                nc.tensor.matmul(
                    cnt_ps[:, :], lhsT, h[:, b, :],
                    start=False, stop=(u == 1 and b == 1),
                )

        # raw extents tile: col 0 = ylo, 1 = xlo, 2 = yhi, 3 = xhi
        # (lo values are lo-512 for hit, 0 for none; hi are hi+1 or 0)
        raw = small.tile([NPC, 4], F32)

        # ---- X extents from cnt_ps [16, 512] on DVE (issued first so
        # the DVE starts X as soon as the stop-matmul lands) ----
        # NOTE: tensor_tensor_reduce and scalar_tensor_tensor (fused DVE
        # ISA ops) both crash the exec unit on this runtime path; use
        # plain compare/mul + reduce.
        colhit = small.tile([NPC, W], F32)
        nc.vector.tensor_scalar(
            colhit[:], cnt_ps[:], 0.5, None, mybir.AluOpType.is_ge
        )
        xprod = scratch.tile([NPC, W], F32, tag="xprod")
        nc.vector.tensor_mul(xprod[:], colhit[:], xlo_c[:])
        nc.vector.tensor_reduce(
            out=raw[:, 1:2], in_=xprod[:],
            axis=mybir.AxisListType.X, op=mybir.AluOpType.min,
        )
        xprod2 = scratch.tile([NPC, W], F32, tag="xprod")
        nc.vector.tensor_mul(xprod2[:], colhit[:], xhi_c[:])
        nc.vector.tensor_reduce(
            out=raw[:, 3:4], in_=xprod2[:],
            axis=mybir.AxisListType.X, op=mybir.AluOpType.max,
        )

        # ---- Y extents in [128, *] space; compare+mul on gpsimd run
        # concurrently with the DVE X chain ----
        rowhit = small.tile([P, NPC * NBLK], F32)
        # on DVE: gpsimd compares run ~17.7ns/elem (1.1us here) and
        # delay the Y muls into DVE's X window; DVE does this in ~190ns
        # during its idle slot right after the last rowmax
        nc.vector.tensor_scalar(
            rowhit[:], rowmax[:], 0.5, None, mybir.AluOpType.is_ge
        )
        # loI[:, i] = min over b of rowhit*(r-512); 0 if no hit (and 0
        # is neutral for the final min since hits give negatives)
        loI = small.tile([P, NPC], F32)
        hiI = small.tile([P, NPC], F32)
        prod = scratch.tile([P, NPC * NBLK], F32, tag="yprod")
        nc.gpsimd.tensor_mul(prod[:], rowhit[:], ylo_c)
        prod_v = prod.rearrange("p (i b) -> p i b", i=NPC)
        nc.vector.tensor_reduce(
            out=loI[:], in_=prod_v[:],
            axis=mybir.AxisListType.X, op=mybir.AluOpType.min,
        )
        prod2 = scratch.tile([P, NPC * NBLK], F32, tag="yprod")
        nc.gpsimd.tensor_mul(prod2[:], rowhit[:], yhi_c)
        prod2_v = prod2.rearrange("p (i b) -> p i b", i=NPC)
        nc.vector.tensor_reduce(
            out=hiI[:], in_=prod2_v[:],
            axis=mybir.AxisListType.X, op=mybir.AluOpType.max,
        )
        # transpose [128, 16] -> [16, 128], partition = image
        nc.tensor.matmul(
            tpsL[:, :], loI[:], ident,
            is_transpose=True, start=True, stop=True,
        )
        nc.tensor.matmul(
            tpsH[:, :], hiI[:], ident,
            is_transpose=True, start=True, stop=True,
        )
        nc.vector.tensor_reduce(
            out=raw[:, 0:1], in_=tpsL[:, :],
            axis=mybir.AxisListType.X, op=mybir.AluOpType.min,
        )
        nc.vector.tensor_reduce(
            out=raw[:, 2:3], in_=tpsH[:, :],
            axis=mybir.AxisListType.X, op=mybir.AluOpType.max,
        )

        # fixup: gm = (hi_raw > 0) * 512 (hit indicator scaled);
        # lo_final = lo_raw + gm   (hit: ymin-512+512 = ymin; none: 0)
        # hi_final = hi_raw + 512 - gm  (hit: hi_raw; none: 512)
        bbox_f = small.tile([NPC, 4], F32)
        gm = small.tile([NPC, 2], F32)
        nc.vector.tensor_scalar(
            gm[:], raw[:, 2:4], 0.0, float(H),
            mybir.AluOpType.is_gt, mybir.AluOpType.mult,
        )
        nc.vector.tensor_add(bbox_f[:, 0:2], raw[:, 0:2], gm[:])
        t5 = small.tile([NPC, 2], F32)
        nc.vector.tensor_scalar_add(t5[:], raw[:, 2:4], float(H))
        nc.vector.tensor_sub(bbox_f[:, 2:4], t5[:], gm[:])

        bbox_i = small.tile([NPC, 4], I32)
        nc.vector.tensor_copy(bbox_i[:], bbox_f[:])
        nc.sync.dma_start(out=bbox_d, in_=bbox_i[:])

    nc.compile()
    return nc


def _consts():
    oneh = np.zeros((P, NPC * NPC), dtype=ml_dtypes.bfloat16)
    for i in range(NPC):
        oneh[:, i * NPC + i] = 1.0
    ident = np.eye(P, dtype=np.float32)
    f = np.arange(W, dtype=np.float32)
    xlo = np.broadcast_to(f - W, (NPC, W)).copy()
    xhi = np.broadcast_to(f + 1, (NPC, W)).copy()
    # block b on partition p is image row r = 4p + b, layout (i b)
    p = np.arange(P)
    b = np.arange(NBLK)
    r = (NBLK * p[:, None] + b[None, :]).astype(np.float32)  # [128, 4]
    ylo = np.tile(r - H, (1, NPC)).astype(np.float32)
    yhi = np.tile(r + 1, (1, NPC)).astype(np.float32)
    pack = np.concatenate([ident, ylo, yhi], axis=1).astype(np.float32)
    return oneh, pack, xlo, xhi


def kernel(mask):
    global _compiled, LAST_RESULTS
    mask = np.ascontiguousarray(np.asarray(mask), dtype=np.float32)
    assert mask.shape == (N, 1, H, W), mask.shape
    if _compiled is None:
        _compiled = _build_nc()
    nc = _compiled
    oneh, pack, xlo, xhi = _consts()
    m = mask.reshape(N, H, W)
    in_maps = []
    for c in range(N_CORES):
        in_maps.append({
            "mask": np.ascontiguousarray(
                m[c * NPC:(c + 1) * NPC].reshape(NPC * H, W)
            ),
            "onehot": oneh,
            "cpack": pack,
            "xlo_const": xlo,
            "xhi_const": xhi,
        })
    res = run_bass_kernel_spmd(nc, in_maps, list(range(N_CORES)), trace=TRACE)
    LAST_RESULTS = res
    out = np.concatenate([res.results[c]["bbox"] for c in range(N_CORES)], axis=0)
    return out.astype(np.int32, copy=False)



# revision 4
# speedup vs baseline: 1.0325x; 1.0325x over previous
"""Bounding-box kernel for Trainium2 (Bass/Tile), 8-core SPMD.

Problem: mask [128, 1, 512, 512] f32 -> bbox [128, 4] int32
  (y_min, x_min, y_max, x_max) of the region where mask >= 0.5,
  with (0, 0, H, W) when a row/col has no hit.

Strategy (per core, 16 images):
  - Stream each image as one [128, 4, 512] DMA (partition p holds rows
    4p..4p+3, contiguous 8KB descriptors). The stream runs at ~420 GB/s
    when nothing stalls the trigger chain, so every per-engine cost must
    stay under the ~2.44 us/image arrival cadence.
  - Threshold on ACT: h = Relu(x*2^34 - (2^33-512)) in bf16, which is
    exactly 0 iff x < 0.5 and >= 512 otherwise (exact for every f32).
    The >=512 scale enables compare-free extents via a min-trick:
       hi_raw = reduce_max(min(mass, idx+1))     (= idx_max+1, or 0)
       lo_raw = reduce_max(min(mass, 512-idx))   (= 512-idx_min, or 0)
  - Column masses: one-hot lhsT matmuls accumulate into PSUM
    (partition = image), split into group A (images 0..12, extents
    computed DURING the stream, own bbox DMA) and group B (13..15,
    short tail chain, 3-row bbox DMA). B gets partition-0-based tiles
    (PSUM reads with a partition offset are rejected by the verifier).
  - Row extents: DVE rowmax per image (bf16 dst) -> [128, 4] slices,
    min-trick against per-partition row-index consts, packed [128,16],
    PE-transposed; A-part early, B-part at the tail.
  - No gpsimd pre-add: PE matmuls pipeline at ~216 ns each, and keeping
    gpsimd/DVE under the cadence is what keeps the DMA stream saturated.
  - Image 15 arrives as two half loads so the final chain is short.
"""

import numpy as np
import ml_dtypes
from contextlib import ExitStack

import concourse.bass as bass
import concourse.bacc as bacc
import concourse.tile as tile
import concourse.mybir as mybir
from concourse.bass_utils import run_bass_kernel_spmd

N_CORES = 8
N, H, W = 128, 512, 512
NPC = N // N_CORES          # images per core = 16
P = 128                     # SBUF partitions
NBLK = H // P               # 4 row blocks per image
F32 = mybir.dt.float32
BF16 = mybir.dt.bfloat16
I32 = mybir.dt.int32

NA = 13                     # images in group A (early extents)
NB = NPC - NA               # images in group B (tail) = 3

# Relu(x * 2^34 - (2^33 - 512)) == 0 iff x < 0.5, >= 512 iff x >= 0.5,
# exact for EVERY f32 x: x*2^34 is exact (power-of-2 scale); for
# x < 0.5, x*2^34 <= 2^33 - 512 so the true sum is <= 0; for x >= 0.5
# the true sum is >= 512 and rounds (f32 then bf16) to >= 512.
ACT_SCALE = float(2**34)
ACT_BIAS = float(512 - 2**33)

TRACE = False               # test.py sets True to capture a HW profile
LAST_RESULTS = None         # BassKernelResults of the last run

_compiled = None


def _build_nc():
    nc = bacc.Bacc(
        "TRN2", target_bir_lowering=False, debug=False, num_devices=N_CORES
    )
    mask_d = nc.dram_tensor("mask", [NPC * H, W], F32, kind="ExternalInput").ap()
    # one-hots: A images as 16-wide slices, then B images as 3-wide slices
    oneh_d = nc.dram_tensor(
        "onehot", [P, NPC * NPC + NB * NB], BF16, kind="ExternalInput"
    ).ap()
    # packed f32 consts: ident [0:128] | yconL [128:132] | yconH [132:136]
    pack_d = nc.dram_tensor("cpack", [P, P + 2 * NBLK], F32, kind="ExternalInput").ap()
    # packed f32 X consts on 16 partitions: xp1 [0:512] | xm512 [512:1024]
    xcon_d = nc.dram_tensor("xcon", [NPC, 2 * W], F32, kind="ExternalInput").ap()
    bbox_d = nc.dram_tensor("bbox", [NPC, 4], I32, kind="ExternalOutput").ap()

    with tile.TileContext(nc) as tc, ExitStack() as ctx:
        consts = ctx.enter_context(tc.tile_pool(name="consts", bufs=1))
        xpool = ctx.enter_context(tc.tile_pool(name="x", bufs=6))
        hpool = ctx.enter_context(tc.tile_pool(name="h", bufs=4))
        lastpool = ctx.enter_context(tc.tile_pool(name="last", bufs=2))
        small = ctx.enter_context(tc.tile_pool(name="small", bufs=1))
        scratch = ctx.enter_context(tc.tile_pool(name="scratch", bufs=2))
        psum = ctx.enter_context(tc.tile_pool(name="psum", bufs=1, space="PSUM"))

        # consts ride the scalar (ACT HWDGE) queue so the sync queue's
        # mask stream descriptors are issued without delay
        with tc.high_priority():
            oneh = consts.tile([P, NPC * NPC + NB * NB], BF16)
            nc.scalar.dma_start(out=oneh[:], in_=oneh_d)
            cpack = consts.tile([P, P + 2 * NBLK], F32)
            nc.scalar.dma_start(out=cpack[:], in_=pack_d)
            xcon = consts.tile([NPC, 2 * W], F32)
            nc.scalar.dma_start(out=xcon[:], in_=xcon_d)
            act_bias = consts.tile([P, 1], F32)
            nc.vector.memset(act_bias[:], ACT_BIAS)
        ident = cpack[:, 0:P]
        yconL = cpack[:, P:P + NBLK]             # [128, 4] = 512 - (4p+b)
        yconH = cpack[:, P + NBLK:P + 2 * NBLK]  # [128, 4] = 4p+b+1
        xp1 = xcon[:, 0:W]                       # [16, 512] = x+1
        xm512 = xcon[:, W:2 * W]                 # [16, 512] = 512-x

        # rowmax[p, i*4 + b]: max over x of h for image row r = 4p + b.
        # bf16 dst (max of bf16 values is exact; 2B dst enables DVE 2x).
        rowmax = small.tile([P, NPC * NBLK], BF16)
        rowmax_v = rowmax.rearrange("p (i b) -> p i b", i=NPC)
        # per-image Y min-trick candidates packed [128, 16] (col = image)
        loP = small.tile([P, NPC], F32)
        hiP = small.tile([P, NPC], F32)
        # column-mass PSUM groups
        cntA = psum.tile([NPC, W], F32)   # images 0..NA-1 (rows 13..15 zero)
        cntB = psum.tile([NB, W], F32)    # images NA..15 on partitions 0..2
        # transposed Y candidate stages
        tpsLA = psum.tile([NA, P], F32)
        tpsHA = psum.tile([NA, P], F32)
        tpsLB = psum.tile([NB, P], F32)
        tpsHB = psum.tile([NB, P], F32)

        # raw extents: col0 = By (512-ymin | 0), col1 = Bx, col2 = Ay
        # (ymax+1 | 0), col3 = Ax.  A rows on partitions 0..12; B group
        # has its own partition-0-based tile.
        rawA = small.tile([NA, 4], F32)
        rawB = small.tile([NB, 4], F32)

        def y_bits(i, h_img):
            """Per-image Y work on DVE: rowmax + min-trick candidates.

            h_img: [P, NBLK, W] bf16 view of the thresholded image.
            """
            nc.vector.tensor_reduce(
                out=rowmax_v[:, i, :], in_=h_img,
                axis=mybir.AxisListType.X, op=mybir.AluOpType.max,
            )
            rmf = scratch.tile([P, NBLK], F32, tag="rmf")
            nc.vector.tensor_copy(rmf[:], rowmax_v[:, i, :])
            cand = scratch.tile([P, 2 * NBLK], F32, tag="ycand")
            nc.vector.tensor_tensor(
                out=cand[:, 0:NBLK], in0=rmf[:], in1=yconL,
                op=mybir.AluOpType.min,
            )
            nc.vector.tensor_tensor(
                out=cand[:, NBLK:2 * NBLK], in0=rmf[:], in1=yconH,
                op=mybir.AluOpType.min,
            )
            cand_v = cand.rearrange("p (s b) -> p s b", s=2)
            nc.vector.tensor_reduce(
                out=loP[:, i:i + 1], in_=cand_v[:, 0:1, :],
                axis=mybir.AxisListType.X, op=mybir.AluOpType.max,
            )
            nc.vector.tensor_reduce(
                out=hiP[:, i:i + 1], in_=cand_v[:, 1:2, :],
                axis=mybir.AxisListType.X, op=mybir.AluOpType.max,
            )

        def x_chain(cnt, nrows, raw, tag):
            """Group X extents: cnt [nrows, W] PSUM -> raw cols 1 and 3."""
            candH = scratch.tile([NPC, W], F32, tag=tag)
            nc.vector.tensor_tensor(
                out=candH[0:nrows, :], in0=cnt[0:nrows, :],
                in1=xp1[0:nrows, :], op=mybir.AluOpType.min,
            )
            nc.vector.tensor_reduce(
                out=raw[0:nrows, 3:4], in_=candH[0:nrows, :],
                axis=mybir.AxisListType.X, op=mybir.AluOpType.max,
            )
            candL = scratch.tile([NPC, W], F32, tag=tag)
            nc.vector.tensor_tensor(
                out=candL[0:nrows, :], in0=cnt[0:nrows, :],
                in1=xm512[0:nrows, :], op=mybir.AluOpType.min,
            )
            nc.vector.tensor_reduce(
                out=raw[0:nrows, 1:2], in_=candL[0:nrows, :],
                axis=mybir.AxisListType.X, op=mybir.AluOpType.max,
            )

        def y_finish(s, nrows, tpsL_t, tpsH_t, raw):
            """Transpose packed Y candidates for images [s, s+nrows)."""
            nc.tensor.matmul(
                tpsL_t[:, :], loP[:, s:s + nrows], ident,
                is_transpose=True, start=True, stop=True,
            )
            nc.tensor.matmul(
                tpsH_t[:, :], hiP[:, s:s + nrows], ident,
                is_transpose=True, start=True, stop=True,
            )
            nc.vector.tensor_reduce(
                out=raw[0:nrows, 0:1], in_=tpsL_t[:, :],
                axis=mybir.AxisListType.X, op=mybir.AluOpType.max,
            )
            nc.vector.tensor_reduce(
                out=raw[0:nrows, 2:3], in_=tpsH_t[:, :],
                axis=mybir.AxisListType.X, op=mybir.AluOpType.max,
            )

        def fixup_and_out(raw, nrows, bbox_slice, tag):
            """raw -> bbox int32 rows + DMA to bbox_slice.

            G = (A_raw > 0) * 512; lo = G - B_raw; hi = A_raw + 512 - G.
            """
            gm = scratch.tile([NPC, 2], F32, tag=tag + "g")
            nc.vector.tensor_scalar(
                gm[0:nrows, :], raw[0:nrows, 2:4], 0.0, float(H),
                mybir.AluOpType.is_gt, mybir.AluOpType.mult,
            )
            bf = scratch.tile([NPC, 4], F32, tag=tag + "f")
            nc.vector.tensor_sub(bf[0:nrows, 0:2], gm[0:nrows, :], raw[0:nrows, 0:2])
            t5 = scratch.tile([NPC, 2], F32, tag=tag + "t")
            nc.vector.tensor_scalar_add(t5[0:nrows, :], raw[0:nrows, 2:4], float(H))
            nc.vector.tensor_sub(bf[0:nrows, 2:4], t5[0:nrows, :], gm[0:nrows, :])
            bi = scratch.tile([NPC, 4], I32, tag=tag + "i")
            nc.vector.tensor_copy(bi[0:nrows, :], bf[0:nrows, :])
            nc.sync.dma_start(out=bbox_slice, in_=bi[0:nrows, :])

        # ---- images 0..14 as single-image DMAs ----
        for i in range(NPC - 1):
            x = xpool.tile([P, NBLK, W], F32, tag="x")
            nc.sync.dma_start(
                out=x[:],
                in_=mask_d[i * H:(i + 1) * H, :]
                .rearrange("(p b) w -> p b w", p=P),
            )
            h = hpool.tile([P, NBLK, W], BF16, tag="h")
            nc.scalar.activation(
                h[:], x[:], mybir.ActivationFunctionType.Relu,
                bias=act_bias[:], scale=ACT_SCALE,
            )
            if i < NA:
                lhsT = oneh[:, i * NPC:(i + 1) * NPC]
                for b in range(NBLK):
                    nc.tensor.matmul(
                        cntA[:, :], lhsT, h[:, b, :],
                        start=(i == 0 and b == 0),
                        stop=(i == NA - 1 and b == NBLK - 1),
                    )
            else:
                j = i - NA
                lhsT = oneh[:, NPC * NPC + j * NB:NPC * NPC + (j + 1) * NB]
                for b in range(NBLK):
                    nc.tensor.matmul(
                        cntB[:, :], lhsT, h[:, b, :],
                        start=(j == 0 and b == 0), stop=False,
                    )
            y_bits(i, h[:])
            if i == NA:
                # group A's matmuls stopped at image NA-1; its X chain,
                # Y transposes, fixup and bbox DMA run here, during the
                # remaining images' streaming
                x_chain(cntA, NA, rawA, "xcandA")
                y_finish(0, NA, tpsLA, tpsHA, rawA)
                fixup_and_out(rawA, NA, bbox_d[0:NA, :], "A")

        # ---- image 15: two half loads -> short tail chain ----
        i = NPC - 1
        j = i - NA
        lhsT = oneh[:, NPC * NPC + j * NB:NPC * NPC + (j + 1) * NB]
        for u in range(2):
            xh = lastpool.tile([P, 2, W], F32, tag="xh")
            nc.sync.dma_start(
                out=xh[:],
                in_=mask_d[i * H:(i + 1) * H, :]
                .rearrange("(p b) w -> p b w", p=P)[:, 2 * u:2 * u + 2, :],
            )
            hh = lastpool.tile([P, 2, W], BF16, tag="hh")
            nc.scalar.activation(
                hh[:], xh[:], mybir.ActivationFunctionType.Relu,
                bias=act_bias[:], scale=ACT_SCALE,
            )
            for b in range(2):
                nc.tensor.matmul(
                    cntB[:, :], lhsT, hh[:, b, :],
                    start=False, stop=(u == 1 and b == 1),
                )
            nc.vector.tensor_reduce(
                out=rowmax_v[:, i, 2 * u:2 * u + 2], in_=hh[:],
                axis=mybir.AxisListType.X, op=mybir.AluOpType.max,
            )

        # image 15 Y candidates (rowmax slices already written)
        rmf15 = scratch.tile([P, NBLK], F32, tag="rmf")
        nc.vector.tensor_copy(rmf15[:], rowmax_v[:, i, :])
        cand15 = scratch.tile([P, 2 * NBLK], F32, tag="ycand")
        nc.vector.tensor_tensor(
            out=cand15[:, 0:NBLK], in0=rmf15[:], in1=yconL,
            op=mybir.AluOpType.min,
        )
        nc.vector.tensor_tensor(
            out=cand15[:, NBLK:2 * NBLK], in0=rmf15[:], in1=yconH,
            op=mybir.AluOpType.min,
        )
        cand15_v = cand15.rearrange("p (s b) -> p s b", s=2)
        nc.vector.tensor_reduce(
            out=loP[:, i:i + 1], in_=cand15_v[:, 0:1, :],
            axis=mybir.AxisListType.X, op=mybir.AluOpType.max,
        )
        nc.vector.tensor_reduce(
            out=hiP[:, i:i + 1], in_=cand15_v[:, 1:2, :],
            axis=mybir.AxisListType.X, op=mybir.AluOpType.max,
        )

        # ---- tail: group B extents + 3-row bbox DMA ----
        x_chain(cntB, NB, rawB, "xcandB")
        y_finish(NA, NB, tpsLB, tpsHB, rawB)
        fixup_and_out(rawB, NB, bbox_d[NA:NPC, :], "B")

    nc.compile()
    return nc


def _consts():
    oneh = np.zeros((P, NPC * NPC + NB * NB), dtype=ml_dtypes.bfloat16)
    for i in range(NPC):
        oneh[:, i * NPC + i] = 1.0      # A layout (only 0..NA-1 used)
    for j in range(NB):
        oneh[:, NPC * NPC + j * NB + j] = 1.0   # B layout
    ident = np.eye(P, dtype=np.float32)
    # block b on partition p is image row r = 4p + b
    p = np.arange(P)
    b = np.arange(NBLK)
    r = (NBLK * p[:, None] + b[None, :]).astype(np.float32)  # [128, 4]
    yconL = (float(H) - r).astype(np.float32)
    yconH = (r + 1.0).astype(np.float32)
    pack = np.concatenate([ident, yconL, yconH], axis=1).astype(np.float32)
    f = np.arange(W, dtype=np.float32)
    xp1 = np.broadcast_to(f + 1.0, (NPC, W))
    xm512 = np.broadcast_to(float(W) - f, (NPC, W))
    xcon = np.concatenate([xp1, xm512], axis=1).astype(np.float32)
    return oneh, pack, xcon


def kernel(mask):
    global _compiled, LAST_RESULTS
    mask = np.ascontiguousarray(np.asarray(mask), dtype=np.float32)
    assert mask.shape == (N, 1, H, W), mask.shape
    if _compiled is None:
        _compiled = _build_nc()
    nc = _compiled
    oneh, pack, xcon = _consts()
    m = mask.reshape(N, H, W)
    in_maps = []
    for c in range(N_CORES):
        in_maps.append({
            "mask": np.ascontiguousarray(
                m[c * NPC:(c + 1) * NPC].reshape(NPC * H, W)
            ),
            "onehot": oneh,
            "cpack": pack,
            "xcon": xcon,
        })
    res = run_bass_kernel_spmd(nc, in_maps, list(range(N_CORES)), trace=TRACE)
    LAST_RESULTS = res
    out = np.concatenate([res.results[c]["bbox"] for c in range(N_CORES)], axis=0)
    return out.astype(np.int32, copy=False)


# revision 10
# speedup vs baseline: 1.0645x; 1.0310x over previous
"""Bounding-box kernel for Trainium2 (Bass/Tile), 8-core SPMD.

Problem: mask [128, 1, 512, 512] f32 -> bbox [128, 4] int32
  (y_min, x_min, y_max, x_max) of the region where mask >= 0.5,
  with (0, 0, H, W) when a row/col has no hit.

Strategy (per core, 16 images):
  - Stream each image as one [128, 4, 512] DMA (partition p holds rows
    4p..4p+3, contiguous 8KB descriptors). The stream runs at ~420 GB/s
    when nothing stalls the trigger chain, so every per-engine cost must
    stay under the ~2.44 us/image arrival cadence.
  - Threshold on ACT: h = Relu(x*2^34 - (2^33-512)) in bf16, which is
    exactly 0 iff x < 0.5 and >= 512 otherwise (exact for every f32).
    The >=512 scale enables compare-free extents via a min-trick:
       hi_raw = reduce_max(min(mass, idx+1))     (= idx_max+1, or 0)
       lo_raw = reduce_max(min(mass, 512-idx))   (= 512-idx_min, or 0)
  - Column masses: one-hot lhsT matmuls accumulate into PSUM
    (partition = image), split into group A (images 0..12, extents
    computed DURING the stream, own bbox DMA) and group B (13..15,
    short tail chain, 3-row bbox DMA). B gets partition-0-based tiles
    (PSUM reads with a partition offset are rejected by the verifier).
  - Row extents: DVE rowmax per image (bf16 dst) -> [128, 4] slices,
    min-trick against per-partition row-index consts, packed [128,16],
    PE-transposed; A-part early, B-part at the tail.
  - No gpsimd pre-add: PE matmuls pipeline at ~216 ns each, and keeping
    gpsimd/DVE under the cadence is what keeps the DMA stream saturated.
  - Image 15 arrives as two half loads so the final chain is short.
"""

import numpy as np
import ml_dtypes
from contextlib import ExitStack

import concourse.bass as bass
import concourse.bacc as bacc
import concourse.tile as tile
import concourse.mybir as mybir
from concourse.bass_utils import run_bass_kernel_spmd

N_CORES = 8
N, H, W = 128, 512, 512
NPC = N // N_CORES          # images per core = 16
P = 128                     # SBUF partitions
NBLK = H // P               # 4 row blocks per image
F32 = mybir.dt.float32
BF16 = mybir.dt.bfloat16
I32 = mybir.dt.int32

NA = 13                     # images in group A (early extents)
NB = NPC - NA               # images in group B (tail) = 3

# Relu(x * 2^34 - (2^33 - 512)) == 0 iff x < 0.5, >= 512 iff x >= 0.5,
# exact for EVERY f32 x: x*2^34 is exact (power-of-2 scale); for
# x < 0.5, x*2^34 <= 2^33 - 512 so the true sum is <= 0; for x >= 0.5
# the true sum is >= 512 and rounds (f32 then bf16) to >= 512.
ACT_SCALE = float(2**34)
ACT_BIAS = float(512 - 2**33)

TRACE = False               # test.py sets True to capture a HW profile
LAST_RESULTS = None         # BassKernelResults of the last run

USE_TTR = False             # tensor_tensor_reduce crashes the exec unit on HW
FOLD_IMAGES = tuple(range(0, NA, 2))   # images whose x-fold runs on gpsimd

_compiled = None


def _build_nc():
    nc = bacc.Bacc(
        "TRN2", target_bir_lowering=False, debug=False, num_devices=N_CORES
    )
    mask_d = nc.dram_tensor("mask", [NPC * H, W], F32, kind="ExternalInput").ap()
    # one-hots: A images as 16-wide slices, then B images as 3-wide slices
    oneh_d = nc.dram_tensor(
        "onehot", [P, NPC * NPC + NB * NB], BF16, kind="ExternalInput"
    ).ap()
    # packed f32 consts: ident [0:128] | yconL [128:132] | yconH [132:136]
    pack_d = nc.dram_tensor("cpack", [P, P + 2 * NBLK], F32, kind="ExternalInput").ap()
    # packed f32 X consts on 16 partitions: xp1 [0:512] | xm512 [512:1024]
    xcon_d = nc.dram_tensor("xcon", [NPC, 2 * W], F32, kind="ExternalInput").ap()
    bbox_d = nc.dram_tensor("bbox", [NPC, 4], I32, kind="ExternalOutput").ap()

    with tile.TileContext(nc) as tc, ExitStack() as ctx:
        consts = ctx.enter_context(tc.tile_pool(name="consts", bufs=1))
        xpool = ctx.enter_context(tc.tile_pool(name="x", bufs=6))
        hpool = ctx.enter_context(tc.tile_pool(name="h", bufs=4))
        lastpool = ctx.enter_context(tc.tile_pool(name="last", bufs=2))
        small = ctx.enter_context(tc.tile_pool(name="small", bufs=1))
        scratch = ctx.enter_context(tc.tile_pool(name="scratch", bufs=2))
        psum = ctx.enter_context(tc.tile_pool(name="psum", bufs=1, space="PSUM"))

        # consts ride the scalar (ACT HWDGE) queue so the sync queue's
        # mask stream descriptors are issued without delay
        with tc.high_priority():
            oneh = consts.tile([P, NPC * NPC + NB * NB], BF16)
            nc.scalar.dma_start(out=oneh[:], in_=oneh_d)
            cpack = consts.tile([P, P + 2 * NBLK], F32)
            nc.scalar.dma_start(out=cpack[:], in_=pack_d)
            xcon = consts.tile([NPC, 2 * W], F32)
            nc.scalar.dma_start(out=xcon[:], in_=xcon_d)
            act_bias = consts.tile([P, 1], F32)
            nc.vector.memset(act_bias[:], ACT_BIAS)
        ident = cpack[:, 0:P]
        yconL = cpack[:, P:P + NBLK]             # [128, 4] = 512 - (4p+b)
        yconH = cpack[:, P + NBLK:P + 2 * NBLK]  # [128, 4] = 4p+b+1
        xp1 = xcon[:, 0:W]                       # [16, 512] = x+1
        xm512 = xcon[:, W:2 * W]                 # [16, 512] = 512-x

        # rowmax[p, i*4 + b]: max over x of h for image row r = 4p + b.
        # bf16 dst (max of bf16 values is exact; 2B dst enables DVE 2x).
        rowmax = small.tile([P, NPC * NBLK], BF16)
        rowmax_v = rowmax.rearrange("p (i b) -> p i b", i=NPC)
        # per-image Y min-trick candidates packed [128, 16] (col = image)
        loP = small.tile([P, NPC], F32)
        hiP = small.tile([P, NPC], F32)
        # column-mass PSUM groups
        cntA = psum.tile([NPC, W], F32)   # images 0..NA-1 (rows 13..15 zero)
        cntB = psum.tile([NB, W], F32)    # images NA..15 on partitions 0..2
        # transposed Y candidate stages
        tpsLA = psum.tile([NA, P], F32)
        tpsHA = psum.tile([NA, P], F32)
        tpsLB = psum.tile([NB, P], F32)
        tpsHB = psum.tile([NB, P], F32)

        # raw extents: col0 = By (512-ymin | 0), col1 = Bx, col2 = Ay
        # (ymax+1 | 0), col3 = Ax.  A rows on partitions 0..12; B group
        # has its own partition-0-based tile.
        rawA = small.tile([NA, 4], F32)
        rawB = small.tile([NB, 4], F32)

        def rowmax_of(i, h_img):
            """rowmax for image i. h_img: [P, NBLK, W] bf16.

            For FOLD_IMAGES the x-fold 512->256 runs on gpsimd (TT max),
            halving the DVE read volume; otherwise DVE reduces directly.
            """
            if i in FOLD_IMAGES:
                # fold by ADD (Pool has no max TT): sum of non-negative
                # masses is still exactly 0 iff no hit, >= 512 otherwise
                hf = scratch.tile([P, NBLK, W // 2], BF16, tag="hf")
                nc.gpsimd.tensor_add(
                    hf[:], h_img[:, :, 0:W // 2], h_img[:, :, W // 2:W]
                )
                nc.vector.tensor_reduce(
                    out=rowmax_v[:, i, :], in_=hf[:],
                    axis=mybir.AxisListType.X, op=mybir.AluOpType.max,
                )
            else:
                nc.vector.tensor_reduce(
                    out=rowmax_v[:, i, :], in_=h_img,
                    axis=mybir.AxisListType.X, op=mybir.AluOpType.max,
                )

        def min_reduce(out_full, accum, in0, in1):
            """accum = reduce_max(min(in0, in1)) — fused if USE_TTR."""
            if USE_TTR:
                nc.vector.tensor_tensor_reduce(
                    out=out_full, in0=in0, in1=in1, scale=1.0, scalar=0.0,
                    op0=mybir.AluOpType.min, op1=mybir.AluOpType.max,
                    accum_out=accum,
                )
            else:
                nc.vector.tensor_tensor(
                    out=out_full, in0=in0, in1=in1, op=mybir.AluOpType.min
                )
                nc.vector.tensor_reduce(
                    out=accum, in_=out_full,
                    axis=mybir.AxisListType.X, op=mybir.AluOpType.max,
                )

        def y_bits(i):
            """Per-image Y min-trick candidates from rowmax_v[:, i, :]."""
            rmf = scratch.tile([P, NBLK], F32, tag="rmf")
            nc.vector.tensor_copy(rmf[:], rowmax_v[:, i, :])
            cand = scratch.tile([P, NBLK], F32, tag="ycand")
            min_reduce(cand[:], loP[:, i:i + 1], rmf[:], yconL)
            cand2 = scratch.tile([P, NBLK], F32, tag="ycand")
            min_reduce(cand2[:], hiP[:, i:i + 1], rmf[:], yconH)

        def x_chain(cnt, nrows, raw, tag):
            """Group X extents: cnt [nrows, W] PSUM -> raw cols 1 and 3."""
            candH = scratch.tile([NPC, W], F32, tag=tag)
            min_reduce(
                candH[0:nrows, :], raw[0:nrows, 3:4],
                cnt[0:nrows, :], xp1[0:nrows, :],
            )
            candL = scratch.tile([NPC, W], F32, tag=tag)
            min_reduce(
                candL[0:nrows, :], raw[0:nrows, 1:2],
                cnt[0:nrows, :], xm512[0:nrows, :],
            )

        def y_finish(s, nrows, tpsL_t, tpsH_t, raw):
            """Transpose packed Y candidates for images [s, s+nrows)."""
            nc.tensor.matmul(
                tpsL_t[:, :], loP[:, s:s + nrows], ident,
                is_transpose=True, start=True, stop=True,
            )
            nc.tensor.matmul(
                tpsH_t[:, :], hiP[:, s:s + nrows], ident,
                is_transpose=True, start=True, stop=True,
            )
            nc.vector.tensor_reduce(
                out=raw[0:nrows, 0:1], in_=tpsL_t[:, :],
                axis=mybir.AxisListType.X, op=mybir.AluOpType.max,
            )
            nc.vector.tensor_reduce(
                out=raw[0:nrows, 2:3], in_=tpsH_t[:, :],
                axis=mybir.AxisListType.X, op=mybir.AluOpType.max,
            )

        def fixup_and_out(raw, nrows, bbox_slice, tag):
            """raw -> bbox int32 rows + DMA to bbox_slice.

            G = (A_raw > 0) * 512; lo = G - B_raw; hi = A_raw + 512 - G.
            """
            gm = scratch.tile([NPC, 2], F32, tag=tag + "g")
            nc.vector.tensor_scalar(
                gm[0:nrows, :], raw[0:nrows, 2:4], 0.0, float(H),
                mybir.AluOpType.is_gt, mybir.AluOpType.mult,
            )
            bf = scratch.tile([NPC, 4], F32, tag=tag + "f")
            nc.vector.tensor_sub(bf[0:nrows, 0:2], gm[0:nrows, :], raw[0:nrows, 0:2])
            t5 = scratch.tile([NPC, 2], F32, tag=tag + "t")
            nc.vector.tensor_scalar_add(t5[0:nrows, :], raw[0:nrows, 2:4], float(H))
            nc.vector.tensor_sub(bf[0:nrows, 2:4], t5[0:nrows, :], gm[0:nrows, :])
            bi = scratch.tile([NPC, 4], I32, tag=tag + "i")
            nc.vector.tensor_copy(bi[0:nrows, :], bf[0:nrows, :])
            nc.sync.dma_start(out=bbox_slice, in_=bi[0:nrows, :])

        # ---- images 0..14 as single-image DMAs ----
        for i in range(NPC - 1):
            x = xpool.tile([P, NBLK, W], F32, tag="x")
            nc.sync.dma_start(
                out=x[:],
                in_=mask_d[i * H:(i + 1) * H, :]
                .rearrange("(p b) w -> p b w", p=P),
            )
            h = hpool.tile([P, NBLK, W], BF16, tag="h")
            nc.scalar.activation(
                h[:], x[:], mybir.ActivationFunctionType.Relu,
                bias=act_bias[:], scale=ACT_SCALE,
            )
            if i < NA:
                lhsT = oneh[:, i * NPC:(i + 1) * NPC]
                for b in range(NBLK):
                    nc.tensor.matmul(
                        cntA[:, :], lhsT, h[:, b, :],
                        start=(i == 0 and b == 0),
                        stop=(i == NA - 1 and b == NBLK - 1),
                    )
            else:
                j = i - NA
                lhsT = oneh[:, NPC * NPC + j * NB:NPC * NPC + (j + 1) * NB]
                for b in range(NBLK):
                    nc.tensor.matmul(
                        cntB[:, :], lhsT, h[:, b, :],
                        start=(j == 0 and b == 0), stop=False,
                    )
            rowmax_of(i, h[:])
            y_bits(i)
            if i == NA:
                # group A's matmuls stopped at image NA-1; its X chain,
                # Y transposes, fixup and bbox DMA run here, during the
                # remaining images' streaming
                x_chain(cntA, NA, rawA, "xcandA")
                y_finish(0, NA, tpsLA, tpsHA, rawA)
                fixup_and_out(rawA, NA, bbox_d[0:NA, :], "A")

        # ---- image 15: two half loads -> short tail chain ----
        i = NPC - 1
        j = i - NA
        lhsT = oneh[:, NPC * NPC + j * NB:NPC * NPC + (j + 1) * NB]
        for u in range(2):
            xh = lastpool.tile([P, 2, W], F32, tag="xh")
            nc.sync.dma_start(
                out=xh[:],
                in_=mask_d[i * H:(i + 1) * H, :]
                .rearrange("(p b) w -> p b w", p=P)[:, 2 * u:2 * u + 2, :],
            )
            hh = lastpool.tile([P, 2, W], BF16, tag="hh")
            nc.scalar.activation(
                hh[:], xh[:], mybir.ActivationFunctionType.Relu,
                bias=act_bias[:], scale=ACT_SCALE,
            )
            for b in range(2):
                nc.tensor.matmul(
                    cntB[:, :], lhsT, hh[:, b, :],
                    start=False, stop=(u == 1 and b == 1),
                )
            nc.vector.tensor_reduce(
                out=rowmax_v[:, i, 2 * u:2 * u + 2], in_=hh[:],
                axis=mybir.AxisListType.X, op=mybir.AluOpType.max,
            )

        # image 15 Y candidates (rowmax slices already written per half)
        y_bits(i)

        # ---- tail: group B extents + 3-row bbox DMA ----
        x_chain(cntB, NB, rawB, "xcandB")
        y_finish(NA, NB, tpsLB, tpsHB, rawB)
        fixup_and_out(rawB, NB, bbox_d[NA:NPC, :], "B")

    nc.compile()
    return nc


def _consts():
    oneh = np.zeros((P, NPC * NPC + NB * NB), dtype=ml_dtypes.bfloat16)
    for i in range(NPC):
        oneh[:, i * NPC + i] = 1.0      # A layout (only 0..NA-1 used)
    for j in range(NB):
        oneh[:, NPC * NPC + j * NB + j] = 1.0   # B layout
    ident = np.eye(P, dtype=np.float32)
    # block b on partition p is image row r = 4p + b
    p = np.arange(P)
    b = np.arange(NBLK)
    r = (NBLK * p[:, None] + b[None, :]).astype(np.float32)  # [128, 4]
    yconL = (float(H) - r).astype(np.float32)
    yconH = (r + 1.0).astype(np.float32)
    pack = np.concatenate([ident, yconL, yconH], axis=1).astype(np.float32)
    f = np.arange(W, dtype=np.float32)
    xp1 = np.broadcast_to(f + 1.0, (NPC, W))
    xm512 = np.broadcast_to(float(W) - f, (NPC, W))
    xcon = np.concatenate([xp1, xm512], axis=1).astype(np.float32)
    return oneh, pack, xcon


def kernel(mask):
    global _compiled, LAST_RESULTS
    mask = np.ascontiguousarray(np.asarray(mask), dtype=np.float32)
    assert mask.shape == (N, 1, H, W), mask.shape
    if _compiled is None:
        _compiled = _build_nc()
    nc = _compiled
    oneh, pack, xcon = _consts()
    m = mask.reshape(N, H, W)
    in_maps = []
    for c in range(N_CORES):
        in_maps.append({
            "mask": np.ascontiguousarray(
                m[c * NPC:(c + 1) * NPC].reshape(NPC * H, W)
            ),
            "onehot": oneh,
            "cpack": pack,
            "xcon": xcon,
        })
    res = run_bass_kernel_spmd(nc, in_maps, list(range(N_CORES)), trace=TRACE)
    LAST_RESULTS = res
    out = np.concatenate([res.results[c]["bbox"] for c in range(N_CORES)], axis=0)
    return out.astype(np.int32, copy=False)
